# revision 14
# baseline (speedup 1.0000x reference)
"""Trainium2 Bass kernel for a 4-layer causal-attention LM.

Model: V=32000, D=1024, H=16 heads, L=4 layers, B=2, S=1024.
  x = emb[tokens] + pos_enc
  per layer: q,k,v = x@W; causal softmax attention; out-proj; residual; LN
  logits = x @ out_w

Sharding over 8 NeuronCores (per sharding hint):
  DP=2 over batch  x  Megatron TP=4 over heads.
  Core c: batch g=c//4, rank r=c%4 owns heads [4r, 4r+4) and vocab cols
  [8000r, 8000(r+1)). Attention/QKV column-parallel, out-proj row-parallel
  with a per-512-column-chunk AllReduce (pipelined). Final vocab projection
  is column-parallel; the host concatenates shards (no collective).

Layout: activations kept feature-major ("xT": [d partitions, seq free]) so
every matmul contracts over partitions with zero transposes (only the
embedding entry needs PE transposes). Matmuls run as float32r (FP22,
full PE rate); the exp/V attention operands run bf16. Softmax uses
transposed scores [sk, sq]; per-query sums come free from a ones column
appended to V in the A@V matmul; normalization folds into the ctx
eviction. LayerNorm stats (feature-axis) via ones-vector matmuls.
"""

import numpy as np

V, D, H, L = 32000, 1024, 16, 4
B, S = 2, 1024
HD = D // H            # 64
P = 128
NG = 4                 # TP degree (cores per batch group)
HL = H // NG           # 4 heads per core
HCOLS = HL * HD        # 256 projection cols per core
VS = V // NG           # 8000 vocab shard
DT = D // P            # 8 d-tiles
SQC = 512              # seq chunk for AR pipelining
NSQC = S // SQC        # 2
NT = S // P            # 8 seq tiles
VC = 500               # vocab tile (8000 = 16*500)
NVC = VS // VC         # 16
SCALE = 1.0 / float(np.sqrt(HD))
EPS = 1e-5
NEG = -1.0e9
RG = [[0, 1, 2, 3], [4, 5, 6, 7]]

_COMPILED = None  # cache (nc) across calls


def _pos_encoding():
    pos = np.arange(S, dtype=np.float32)[:, None]
    div = np.exp(np.arange(0, D, 2, dtype=np.float32) * (-np.log(10000.0) / D))
    ang = pos * div
    pe = np.stack([np.sin(ang), np.cos(ang)], axis=-1).reshape(S, D)
    return pe.astype(np.float32)


def _build():
    import concourse.bass as bass
    import concourse.tile as tile
    from concourse import bacc, mybir

    f32 = mybir.dt.float32
    f32r = mybir.dt.float32r
    bf16 = mybir.dt.bfloat16
    i32 = mybir.dt.int32
    AF = mybir.ActivationFunctionType

    nc = bacc.Bacc("TRN2", target_bir_lowering=False, debug=False, num_devices=8)

    tok = nc.dram_tensor("tok", [S, 1], i32, kind="ExternalInput").ap()
    ident_d = nc.dram_tensor("ident", [P, P], f32, kind="ExternalInput").ap()
    ones_d = nc.dram_tensor("onesc", [P, 1], f32r, kind="ExternalInput").ap()
    masks_d = nc.dram_tensor("masks", [P, 4 * SQC], f32, kind="ExternalInput").ap()
    emb = nc.dram_tensor("emb", [V, D], f32, kind="ExternalInput").ap()
    peT = nc.dram_tensor("peT", [D, S], f32, kind="ExternalInput").ap()
    qw = nc.dram_tensor("qw", [L, D, HCOLS], f32r, kind="ExternalInput").ap()
    kw = nc.dram_tensor("kw", [L, D, HCOLS], f32r, kind="ExternalInput").ap()
    vw = nc.dram_tensor("vw", [L, D, HCOLS], f32r, kind="ExternalInput").ap()
    ow = nc.dram_tensor("ow", [L, HCOLS, D], f32r, kind="ExternalInput").ap()
    outw = nc.dram_tensor("outw", [D, VS], f32r, kind="ExternalInput").ap()
    out = nc.dram_tensor("out", [S, VS], f32, kind="ExternalOutput").ap()

    with tile.TileContext(nc) as tc:
        with (
            tc.tile_pool(name="const", bufs=1) as constp,
            tc.tile_pool(name="xp", bufs=1) as xp,
            tc.tile_pool(name="psum", bufs=2, space="PSUM") as psp,
        ):
            # ---- constants (host-provided: walrus chokes on affine_select) ----
            ident = constp.tile([P, P], f32)
            nc.sync.dma_start(out=ident[:], in_=ident_d[:])
            ones = constp.tile([P, 1], f32r)
            nc.sync.dma_start(out=ones[:], in_=ones_d[:])
            epsb = constp.tile([1, 1], f32)
            nc.vector.memset(epsb[:], EPS)
            # additive causal masks for the 4 diagonal sk-tiles of each sq
            # chunk: mask[trel][i, j] = 0 if j >= 128*trel + i else NEG
            masks = constp.tile([P, 4, SQC], f32)
            nc.sync.dma_start(
                out=masks[:], in_=masks_d.rearrange("p (t s) -> p t s", t=4)
            )

            # persistent activations, feature-major: x[d, s], d = a*128 + p
            xT = xp.tile([P, DT, S], f32r)

            # ---- embedding: gather rows, transpose to feature-major, +pe ----
            with tc.tile_pool(name="embp", bufs=2) as embp:
                tokt = embp.tile([P, NT], i32, bufs=1)
                nc.sync.dma_start(
                    out=tokt[:], in_=tok.rearrange("(t p) o -> p (t o)", p=P)
                )
                for st in range(NT):
                    xrow = embp.tile([P, D], f32, tag="xrow")
                    nc.gpsimd.indirect_dma_start(
                        out=xrow[:],
                        out_offset=None,
                        in_=emb[:],
                        in_offset=bass.IndirectOffsetOnAxis(
                            ap=tokt[:, st : st + 1], axis=0
                        ),
                    )
                    pesb = embp.tile([P, DT, P], f32, tag="pesb")
                    nc.sync.dma_start(
                        out=pesb[:],
                        in_=peT[:, st * P : (st + 1) * P].rearrange(
                            "(a p) s -> p a s", p=P
                        ),
                    )
                    for dc in range(DT):
                        tps = psp.tile([P, P], f32, tag="mm", name=f"tps_{st}_{dc}")
                        nc.tensor.transpose(
                            tps[:], xrow[:, dc * P : (dc + 1) * P], ident[:]
                        )
                        nc.vector.tensor_add(
                            xT[:, dc, st * P : (st + 1) * P],
                            tps[:],
                            pesb[:, dc, :],
                        )

            # ---- transformer layers ----
            with (
                tc.tile_pool(name="wp", bufs=4) as wp,
                tc.tile_pool(name="owp", bufs=2) as owp,
                tc.tile_pool(name="apl", bufs=1) as apool,
                tc.tile_pool(name="expp", bufs=4) as expp,
                tc.tile_pool(name="lnp", bufs=1) as lnp,
                tc.tile_pool(name="dcp", bufs=2) as dcp,
                tc.tile_pool(name="small", bufs=1) as smallp,
                tc.tile_pool(name="dram", bufs=2, space="DRAM") as dramp,
            ):
                for l in range(L):
                    qw_sb = wp.tile([P, DT, HCOLS], f32r, tag="w", name=f"qw{l}")
                    nc.sync.dma_start(
                        out=qw_sb[:], in_=qw[l].rearrange("(a p) m -> p a m", p=P)
                    )
                    kw_sb = wp.tile([P, DT, HCOLS], f32r, tag="w", name=f"kw{l}")
                    nc.sync.dma_start(
                        out=kw_sb[:], in_=kw[l].rearrange("(a p) m -> p a m", p=P)
                    )
                    vw_sb = wp.tile([P, DT, HCOLS], f32r, tag="w", name=f"vw{l}")
                    nc.sync.dma_start(
                        out=vw_sb[:], in_=vw[l].rearrange("(a p) m -> p a m", p=P)
                    )
                    ow_sb = owp.tile([P, 2, D], f32r, tag="ow", name=f"ow{l}")
                    nc.sync.dma_start(
                        out=ow_sb[:], in_=ow[l].rearrange("(a p) m -> p a m", p=P)
                    )

                    # q,k feature-major [headcol, s]; head h: partitions
                    # 64*(h%2).., chunk h//2
                    qT = apool.tile([P, 2, S], f32r, tag="qT", name=f"qT{l}")
                    kT = apool.tile([P, 2, S], f32r, tag="kT", name=f"kT{l}")
                    for dst, wsb in ((qT, qw_sb), (kT, kw_sb)):
                        for hp in range(2):
                            for c in range(NSQC):
                                ps = psp.tile([P, SQC], f32, tag="mm")
                                for kt in range(DT):
                                    nc.tensor.matmul(
                                        ps[:],
                                        lhsT=wsb[:, kt, hp * P : (hp + 1) * P],
                                        rhs=xT[:, kt, c * SQC : (c + 1) * SQC],
                                        start=(kt == 0),
                                        stop=(kt == DT - 1),
                                    )
                                nc.scalar.copy(
                                    dst[:, hp, c * SQC : (c + 1) * SQC], ps[:]
                                )

                    # v seq-major [s, headcol] bf16, with ones column at 64
                    vS = apool.tile([P, NT, HL, 66], bf16, tag="vS", name=f"vS{l}")
                    for st in range(NT):
                        nc.vector.memset(vS[:, st, :, 64:65], 1.0)
                    for st in range(NT):
                        ps = psp.tile([P, HCOLS], f32, tag="mm")
                        for kt in range(DT):
                            nc.tensor.matmul(
                                ps[:],
                                lhsT=xT[:, kt, st * P : (st + 1) * P],
                                rhs=vw_sb[:, kt, :],
                                start=(kt == 0),
                                stop=(kt == DT - 1),
                            )
                        nc.scalar.copy(
                            vS[:, st, :, 0:64],
                            ps[:].rearrange("p (h e) -> p h e", h=HL),
                        )

                    # ---- attention (transposed scores), ctx feature-major ----
                    ctx = apool.tile([P, 2, S], f32r, tag="ctx", name=f"ctx{l}")
                    for h in range(HL):
                        hp, hr = divmod(h, 2)
                        p0 = 64 * hr
                        for c in range(NSQC):
                            nt_vis = 4 * c + 4
                            av = psp.tile([P, SQC], f32, tag="av")
                            for t in range(nt_vis):
                                sc = psp.tile([P, SQC], f32, tag="sc")
                                nc.tensor.matmul(
                                    sc[:],
                                    lhsT=kT[p0 : p0 + 64, hp, t * P : (t + 1) * P],
                                    rhs=qT[p0 : p0 + 64, hp, c * SQC : (c + 1) * SQC],
                                    start=True,
                                    stop=True,
                                )
                                trel = t - 4 * c
                                if trel >= 0:
                                    nc.vector.tensor_add(
                                        sc[:], sc[:], masks[:, trel, :]
                                    )
                                ex = expp.tile([P, SQC], bf16, tag="ex")
                                nc.scalar.activation(
                                    ex[:], sc[:], AF.Exp, scale=SCALE
                                )
                                nc.tensor.matmul(
                                    av[0:65, :],
                                    lhsT=vS[:, t, h, 0:65],
                                    rhs=ex[:],
                                    start=(t == 0),
                                    stop=(t == nt_vis - 1),
                                )
                            inv = smallp.tile([1, SQC], f32, tag="inv")
                            nc.vector.reciprocal(inv[:], av[64:65, :])
                            invb = smallp.tile([64, SQC], f32, tag="invb")
                            nc.gpsimd.partition_broadcast(invb[:], inv[:])
                            nc.vector.tensor_mul(
                                ctx[p0 : p0 + 64, hp, c * SQC : (c + 1) * SQC],
                                av[0:64, :],
                                invb[:],
                            )

                    # ---- out-proj partial + AR + residual + LN (per chunk) ----
                    for c in range(NSQC):
                        ar_in = dramp.tile(
                            [D, SQC], f32, tag="arin", name=f"ari{l}_{c}"
                        )
                        for dc in range(DT):
                            ps = psp.tile([P, SQC], f32, tag="mm")
                            for kt in range(2):
                                nc.tensor.matmul(
                                    ps[:],
                                    lhsT=ow_sb[:, kt, dc * P : (dc + 1) * P],
                                    rhs=ctx[:, kt, c * SQC : (c + 1) * SQC],
                                    start=(kt == 0),
                                    stop=(kt == 1),
                                )
                            po = dcp.tile([P, SQC], f32, tag="po")
                            nc.scalar.copy(po[:], ps[:])
                            nc.sync.dma_start(
                                out=ar_in[dc * P : (dc + 1) * P, :], in_=po[:]
                            )
                        ar_out = dramp.tile(
                            [D, SQC], f32, tag="arout", name=f"aro{l}_{c}"
                        )
                        nc.gpsimd.collective_compute(
                            "AllReduce",
                            mybir.AluOpType.add,
                            replica_groups=RG,
                            ins=[ar_in[:].opt()],
                            outs=[ar_out[:].opt()],
                        )
                        # residual + stats + normalize (in place into xT)
                        xr = lnp.tile([P, DT, SQC], f32r, tag="xr")
                        st0 = psp.tile([1, SQC], f32, tag="av")
                        st1 = psp.tile([1, SQC], f32, tag="sc")
                        for dc in range(DT):
                            arL = dcp.tile([P, SQC], f32, tag="arL")
                            nc.sync.dma_start(
                                out=arL[:], in_=ar_out[dc * P : (dc + 1) * P, :]
                            )
                            nc.vector.tensor_add(
                                xr[:, dc, :],
                                arL[:],
                                xT[:, dc, c * SQC : (c + 1) * SQC],
                            )
                            sqt = dcp.tile([P, SQC], f32r, tag="sqt")
                            nc.scalar.activation(sqt[:], xr[:, dc, :], AF.Square)
                            nc.tensor.matmul(
                                st0[:],
                                lhsT=ones[:],
                                rhs=xr[:, dc, :],
                                start=(dc == 0),
                                stop=(dc == DT - 1),
                            )
                            nc.tensor.matmul(
                                st1[:],
                                lhsT=ones[:],
                                rhs=sqt[:],
                                start=(dc == 0),
                                stop=(dc == DT - 1),
                            )
                        mean = smallp.tile([1, SQC], f32, tag="mean")
                        nc.scalar.mul(mean[:], st0[:], 1.0 / D)
                        nmean = smallp.tile([1, SQC], f32, tag="nmean")
                        nc.scalar.mul(nmean[:], mean[:], -1.0)
                        msq = smallp.tile([1, SQC], f32, tag="msq")
                        nc.scalar.activation(msq[:], mean[:], AF.Square)
                        ex2 = smallp.tile([1, SQC], f32, tag="ex2")
                        nc.scalar.mul(ex2[:], st1[:], 1.0 / D)
                        var = smallp.tile([1, SQC], f32, tag="var")
                        nc.vector.tensor_sub(var[:], ex2[:], msq[:])
                        std = smallp.tile([1, SQC], f32, tag="std")
                        nc.scalar.activation(std[:], var[:], AF.Sqrt, bias=epsb[:])
                        rstd = smallp.tile([1, SQC], f32, tag="rstd")
                        nc.vector.reciprocal(rstd[:], std[:])
                        mb = smallp.tile([P, SQC], f32, tag="mb")
                        nc.gpsimd.partition_broadcast(mb[:], nmean[:])
                        rb = smallp.tile([P, SQC], f32, tag="rb")
                        nc.gpsimd.partition_broadcast(rb[:], rstd[:])
                        for dc in range(DT):
                            nc.vector.tensor_add(xr[:, dc, :], xr[:, dc, :], mb[:])
                            nc.vector.tensor_mul(
                                xT[:, dc, c * SQC : (c + 1) * SQC],
                                xr[:, dc, :],
                                rb[:],
                            )

            # ---- final vocab projection (column-parallel, no collective) ----
            with (
                tc.tile_pool(name="fw", bufs=2) as fwp,
                tc.tile_pool(name="ob", bufs=4) as obp,
            ):
                for vc in range(NVC):
                    wv = fwp.tile([P, DT, VC], f32r, tag="wv")
                    nc.sync.dma_start(
                        out=wv[:],
                        in_=outw[:, vc * VC : (vc + 1) * VC].rearrange(
                            "(a p) m -> p a m", p=P
                        ),
                    )
                    for st in range(NT):
                        ps = psp.tile([P, SQC], f32, tag="mm")
                        for kt in range(DT):
                            nc.tensor.matmul(
                                ps[:, 0:VC],
                                lhsT=xT[:, kt, st * P : (st + 1) * P],
                                rhs=wv[:, kt, :],
                                start=(kt == 0),
                                stop=(kt == DT - 1),
                            )
                        ob = obp.tile([P, VC], f32, tag="ob")
                        nc.scalar.copy(ob[:], ps[:, 0:VC])
                        nc.sync.dma_start(
                            out=out[st * P : (st + 1) * P, vc * VC : (vc + 1) * VC],
                            in_=ob[:],
                        )
    nc.finalize()
    return nc


def _in_maps(tokens, emb, qw, kw, vw, ow, out_w):
    pe = _pos_encoding()
    peT = np.ascontiguousarray(pe.T)
    ident = np.eye(P, dtype=np.float32)
    j = np.arange(SQC)[None, :]
    i = np.arange(P)[:, None]
    masks = np.concatenate(
        [
            np.where(j >= P * trel + i, 0.0, NEG).astype(np.float32)
            for trel in range(4)
        ],
        axis=1,
    )
    masks = np.ascontiguousarray(masks)
    maps = []
    for c in range(8):
        g, r = divmod(c, NG)
        hc0 = r * HCOLS
        maps.append(
            {
                "tok": np.ascontiguousarray(
                    tokens[g].reshape(S, 1).astype(np.int32)
                ),
                "ident": ident,
                "onesc": np.ones((P, 1), dtype=np.float32),
                "masks": masks,
                "emb": emb,
                "peT": peT,
                "qw": np.ascontiguousarray(qw[:, :, hc0 : hc0 + HCOLS]),
                "kw": np.ascontiguousarray(kw[:, :, hc0 : hc0 + HCOLS]),
                "vw": np.ascontiguousarray(vw[:, :, hc0 : hc0 + HCOLS]),
                "ow": np.ascontiguousarray(ow[:, hc0 : hc0 + HCOLS, :]),
                "outw": np.ascontiguousarray(out_w[:, r * VS : (r + 1) * VS]),
            }
        )
    return maps


def run(inputs, trace=False):
    """Build+compile (cached), run on 8 cores, return (full_output, results)."""
    global _COMPILED
    from concourse.bass_utils import run_bass_kernel_spmd

    if _COMPILED is None:
        _COMPILED = _build()
    nc = _COMPILED

    tokens = np.asarray(inputs["tokens"])
    maps = _in_maps(
        np.asarray(tokens),
        np.ascontiguousarray(np.asarray(inputs["emb"], dtype=np.float32)),
        np.asarray(inputs["qw"], dtype=np.float32),
        np.asarray(inputs["kw"], dtype=np.float32),
        np.asarray(inputs["vw"], dtype=np.float32),
        np.asarray(inputs["ow"], dtype=np.float32),
        np.ascontiguousarray(np.asarray(inputs["out_w"], dtype=np.float32)),
    )
    res = run_bass_kernel_spmd(nc, maps, core_ids=list(range(8)), trace=trace)
    full = np.empty((B, S, V), dtype=np.float32)
    for c in range(8):
        g, r = divmod(c, NG)
        full[g, :, r * VS : (r + 1) * VS] = res.results[c]["out"]
    return full, res


def kernel(**inputs):
    full, _ = run(inputs)
    return full


# revision 15
# speedup vs baseline: 1.1738x; 1.1738x over previous
"""Trainium2 Bass kernel for a 4-layer causal-attention LM.

Model: V=32000, D=1024, H=16 heads, L=4 layers, B=2, S=1024.
  x = emb[tokens] + pos_enc
  per layer: q,k,v = x@W; causal softmax attention; out-proj; residual; LN
  logits = x @ out_w

Sharding over 8 NeuronCores (per sharding hint):
  DP=2 over batch  x  Megatron TP=4 over heads.
  Core c: batch g=c//4, rank r=c%4 owns heads [4r, 4r+4) and vocab cols
  [8000r, 8000(r+1)). Attention/QKV column-parallel, out-proj row-parallel
  with a per-512-column-chunk AllReduce (pipelined). Final vocab projection
  is column-parallel; the host concatenates shards (no collective).

Layout: activations kept feature-major ("xT": [d partitions, seq free]) so
every matmul contracts over partitions with zero transposes (only the
embedding entry needs PE transposes). Matmuls run as float32r (FP22,
full PE rate); the exp/V attention operands run bf16. Softmax uses
transposed scores [sk, sq]; per-query sums come free from a ones column
appended to V in the A@V matmul; normalization folds into the ctx
eviction. LayerNorm stats (feature-axis) via ones-vector matmuls.
"""

import numpy as np

V, D, H, L = 32000, 1024, 16, 4
B, S = 2, 1024
HD = D // H            # 64
P = 128
NG = 4                 # TP degree (cores per batch group)
HL = H // NG           # 4 heads per core
HCOLS = HL * HD        # 256 projection cols per core
VS = V // NG           # 8000 vocab shard
DT = D // P            # 8 d-tiles
SQC = 512              # seq chunk for AR pipelining
NSQC = S // SQC        # 2
NT = S // P            # 8 seq tiles
VC = 500               # vocab tile (8000 = 16*500)
NVC = VS // VC         # 16
SCALE = 1.0 / float(np.sqrt(HD))
EPS = 1e-5
NEG = -1.0e9
RG = [[0, 1, 2, 3], [4, 5, 6, 7]]

_COMPILED = None  # cache (nc) across calls


def _pos_encoding():
    pos = np.arange(S, dtype=np.float32)[:, None]
    div = np.exp(np.arange(0, D, 2, dtype=np.float32) * (-np.log(10000.0) / D))
    ang = pos * div
    pe = np.stack([np.sin(ang), np.cos(ang)], axis=-1).reshape(S, D)
    return pe.astype(np.float32)


def _build():
    import concourse.bass as bass
    import concourse.tile as tile
    from concourse import bacc, mybir

    f32 = mybir.dt.float32
    f32r = mybir.dt.float32r
    bf16 = mybir.dt.bfloat16
    i32 = mybir.dt.int32
    AF = mybir.ActivationFunctionType

    nc = bacc.Bacc("TRN2", target_bir_lowering=False, debug=False, num_devices=8)

    tok = nc.dram_tensor("tok", [S, 1], i32, kind="ExternalInput").ap()
    ident_d = nc.dram_tensor("ident", [P, P], f32, kind="ExternalInput").ap()
    ones_d = nc.dram_tensor("onesc", [P, 1], f32r, kind="ExternalInput").ap()
    masks_d = nc.dram_tensor("masks", [P, 4 * SQC], f32, kind="ExternalInput").ap()
    emb = nc.dram_tensor("emb", [V, D], f32, kind="ExternalInput").ap()
    peT = nc.dram_tensor("peT", [D, S], f32, kind="ExternalInput").ap()
    qw = nc.dram_tensor("qw", [L, D, HCOLS], f32r, kind="ExternalInput").ap()
    kw = nc.dram_tensor("kw", [L, D, HCOLS], f32r, kind="ExternalInput").ap()
    vw = nc.dram_tensor("vw", [L, D, HCOLS], f32r, kind="ExternalInput").ap()
    ow = nc.dram_tensor("ow", [L, HCOLS, D], f32r, kind="ExternalInput").ap()
    outw = nc.dram_tensor("outw", [D, VS], f32r, kind="ExternalInput").ap()
    out = nc.dram_tensor("out", [S, VS], f32, kind="ExternalOutput").ap()

    with tile.TileContext(nc) as tc:
        with (
            tc.tile_pool(name="const", bufs=1) as constp,
            tc.tile_pool(name="xp", bufs=1) as xp,
            tc.tile_pool(name="psum", bufs=2, space="PSUM") as psp,
        ):
            # ---- constants (host-provided: walrus chokes on affine_select) ----
            ident = constp.tile([P, P], f32)
            nc.sync.dma_start(out=ident[:], in_=ident_d[:])
            ones = constp.tile([P, 1], f32r)
            nc.sync.dma_start(out=ones[:], in_=ones_d[:])
            epsb = constp.tile([1, 1], f32)
            nc.vector.memset(epsb[:], EPS)
            # additive causal masks for the 4 diagonal sk-tiles of each sq
            # chunk: mask[trel][i, j] = 0 if j >= 128*trel + i else NEG
            masks = constp.tile([P, 4, SQC], f32)
            nc.sync.dma_start(
                out=masks[:], in_=masks_d.rearrange("p (t s) -> p t s", t=4)
            )

            # persistent activations, feature-major: x[d, s], d = a*128 + p
            xT = xp.tile([P, DT, S], f32r)

            # ---- embedding: gather rows, transpose to feature-major, +pe ----
            with tc.tile_pool(name="embp", bufs=2) as embp:
                tokt = embp.tile([P, NT], i32, bufs=1)
                nc.sync.dma_start(
                    out=tokt[:], in_=tok.rearrange("(t p) o -> p (t o)", p=P)
                )
                for st in range(NT):
                    xrow = embp.tile([P, D], f32, tag="xrow")
                    nc.gpsimd.indirect_dma_start(
                        out=xrow[:],
                        out_offset=None,
                        in_=emb[:],
                        in_offset=bass.IndirectOffsetOnAxis(
                            ap=tokt[:, st : st + 1], axis=0
                        ),
                    )
                    pesb = embp.tile([P, DT, P], f32, tag="pesb")
                    nc.sync.dma_start(
                        out=pesb[:],
                        in_=peT[:, st * P : (st + 1) * P].rearrange(
                            "(a p) s -> p a s", p=P
                        ),
                    )
                    for dc in range(DT):
                        tps = psp.tile([P, P], f32, tag="mm", name=f"tps_{st}_{dc}")
                        nc.tensor.transpose(
                            tps[:], xrow[:, dc * P : (dc + 1) * P], ident[:]
                        )
                        nc.vector.tensor_add(
                            xT[:, dc, st * P : (st + 1) * P],
                            tps[:],
                            pesb[:, dc, :],
                        )

            # ---- transformer layers ----
            with (
                tc.tile_pool(name="wp", bufs=4) as wp,
                tc.tile_pool(name="owp", bufs=2) as owp,
                tc.tile_pool(name="apl", bufs=1) as apool,
                tc.tile_pool(name="expp", bufs=4) as expp,
                tc.tile_pool(name="lnp", bufs=1) as lnp,
                tc.tile_pool(name="dcp", bufs=2) as dcp,
                tc.tile_pool(name="small", bufs=1) as smallp,
                tc.tile_pool(name="dram", bufs=2, space="DRAM") as dramp,
            ):
                for l in range(L):
                    qw_sb = wp.tile([P, DT, HCOLS], f32r, tag="w", name=f"qw{l}")
                    nc.sync.dma_start(
                        out=qw_sb[:], in_=qw[l].rearrange("(a p) m -> p a m", p=P)
                    )
                    kw_sb = wp.tile([P, DT, HCOLS], f32r, tag="w", name=f"kw{l}")
                    nc.sync.dma_start(
                        out=kw_sb[:], in_=kw[l].rearrange("(a p) m -> p a m", p=P)
                    )
                    vw_sb = wp.tile([P, DT, HCOLS], f32r, tag="w", name=f"vw{l}")
                    nc.sync.dma_start(
                        out=vw_sb[:], in_=vw[l].rearrange("(a p) m -> p a m", p=P)
                    )
                    ow_sb = owp.tile([P, 2, D], f32r, tag="ow", name=f"ow{l}")
                    nc.sync.dma_start(
                        out=ow_sb[:], in_=ow[l].rearrange("(a p) m -> p a m", p=P)
                    )

                    # q,k feature-major [headcol, s]; head h: partitions
                    # 64*(h%2).., chunk h//2
                    qT = apool.tile([P, 2, S], f32r, tag="qT", name=f"qT{l}")
                    kT = apool.tile([P, 2, S], f32r, tag="kT", name=f"kT{l}")
                    for dst, wsb in ((qT, qw_sb), (kT, kw_sb)):
                        for hp in range(2):
                            for c in range(NSQC):
                                ps = psp.tile([P, SQC], f32, tag="mm")
                                for kt in range(DT):
                                    nc.tensor.matmul(
                                        ps[:],
                                        lhsT=wsb[:, kt, hp * P : (hp + 1) * P],
                                        rhs=xT[:, kt, c * SQC : (c + 1) * SQC],
                                        start=(kt == 0),
                                        stop=(kt == DT - 1),
                                    )
                                nc.scalar.copy(
                                    dst[:, hp, c * SQC : (c + 1) * SQC], ps[:]
                                )

                    # v seq-major [s, headcol] bf16, with ones column at 64
                    vS = apool.tile([P, NT, HL, 66], bf16, tag="vS", name=f"vS{l}")
                    for st in range(NT):
                        nc.vector.memset(vS[:, st, :, 64:65], 1.0)
                    for st in range(NT):
                        ps = psp.tile([P, HCOLS], f32, tag="mm")
                        for kt in range(DT):
                            nc.tensor.matmul(
                                ps[:],
                                lhsT=xT[:, kt, st * P : (st + 1) * P],
                                rhs=vw_sb[:, kt, :],
                                start=(kt == 0),
                                stop=(kt == DT - 1),
                            )
                        nc.scalar.copy(
                            vS[:, st, :, 0:64],
                            ps[:].rearrange("p (h e) -> p h e", h=HL),
                        )

                    # ---- attention (transposed scores), ctx feature-major ----
                    ctx = apool.tile([P, 2, S], f32r, tag="ctx", name=f"ctx{l}")
                    for h in range(HL):
                        hp, hr = divmod(h, 2)
                        p0 = 64 * hr
                        for c in range(NSQC):
                            nt_vis = 4 * c + 4
                            av = psp.tile([P, SQC], f32, tag="av")
                            for t in range(nt_vis):
                                sc = psp.tile([P, SQC], f32, tag="sc")
                                nc.tensor.matmul(
                                    sc[:],
                                    lhsT=kT[p0 : p0 + 64, hp, t * P : (t + 1) * P],
                                    rhs=qT[p0 : p0 + 64, hp, c * SQC : (c + 1) * SQC],
                                    start=True,
                                    stop=True,
                                )
                                trel = t - 4 * c
                                if trel >= 0:
                                    nc.vector.tensor_add(
                                        sc[:], sc[:], masks[:, trel, :]
                                    )
                                ex = expp.tile([P, SQC], bf16, tag="ex")
                                nc.scalar.activation(
                                    ex[:], sc[:], AF.Exp, scale=SCALE
                                )
                                nc.tensor.matmul(
                                    av[0:65, :],
                                    lhsT=vS[:, t, h, 0:65],
                                    rhs=ex[:],
                                    start=(t == 0),
                                    stop=(t == nt_vis - 1),
                                )
                            inv = smallp.tile([1, SQC], f32, tag="inv")
                            nc.vector.reciprocal(inv[:], av[64:65, :])
                            invb = smallp.tile([64, SQC], f32, tag="invb")
                            nc.gpsimd.partition_broadcast(invb[:], inv[:])
                            nc.vector.tensor_mul(
                                ctx[p0 : p0 + 64, hp, c * SQC : (c + 1) * SQC],
                                av[0:64, :],
                                invb[:],
                            )

                    # ---- out-proj partial + AR + residual + LN (per chunk) ----
                    for c in range(NSQC):
                        ar_in = dramp.tile(
                            [D, SQC], bf16, tag="arin", name=f"ari{l}_{c}"
                        )
                        for dc in range(DT):
                            ps = psp.tile([P, SQC], f32, tag="mm")
                            for kt in range(2):
                                nc.tensor.matmul(
                                    ps[:],
                                    lhsT=ow_sb[:, kt, dc * P : (dc + 1) * P],
                                    rhs=ctx[:, kt, c * SQC : (c + 1) * SQC],
                                    start=(kt == 0),
                                    stop=(kt == 1),
                                )
                            po = dcp.tile([P, SQC], bf16, tag="po")
                            nc.scalar.copy(po[:], ps[:])
                            nc.sync.dma_start(
                                out=ar_in[dc * P : (dc + 1) * P, :], in_=po[:]
                            )
                        ar_out = dramp.tile(
                            [D, SQC], bf16, tag="arout", name=f"aro{l}_{c}"
                        )
                        nc.gpsimd.collective_compute(
                            "AllReduce",
                            mybir.AluOpType.add,
                            replica_groups=RG,
                            ins=[ar_in[:].opt()],
                            outs=[ar_out[:].opt()],
                        )
                        # residual + stats + normalize (in place into xT)
                        xr = lnp.tile([P, DT, SQC], f32r, tag="xr")
                        st0 = psp.tile([1, SQC], f32, tag="av")
                        st1 = psp.tile([1, SQC], f32, tag="sc")
                        for dc in range(DT):
                            arL = dcp.tile([P, SQC], bf16, tag="arL")
                            nc.sync.dma_start(
                                out=arL[:], in_=ar_out[dc * P : (dc + 1) * P, :]
                            )
                            nc.vector.tensor_add(
                                xr[:, dc, :],
                                arL[:],
                                xT[:, dc, c * SQC : (c + 1) * SQC],
                            )
                            sqt = dcp.tile([P, SQC], f32r, tag="sqt")
                            nc.scalar.activation(sqt[:], xr[:, dc, :], AF.Square)
                            nc.tensor.matmul(
                                st0[:],
                                lhsT=ones[:],
                                rhs=xr[:, dc, :],
                                start=(dc == 0),
                                stop=(dc == DT - 1),
                            )
                            nc.tensor.matmul(
                                st1[:],
                                lhsT=ones[:],
                                rhs=sqt[:],
                                start=(dc == 0),
                                stop=(dc == DT - 1),
                            )
                        mean = smallp.tile([1, SQC], f32, tag="mean")
                        nc.scalar.mul(mean[:], st0[:], 1.0 / D)
                        nmean = smallp.tile([1, SQC], f32, tag="nmean")
                        nc.scalar.mul(nmean[:], mean[:], -1.0)
                        msq = smallp.tile([1, SQC], f32, tag="msq")
                        nc.scalar.activation(msq[:], mean[:], AF.Square)
                        ex2 = smallp.tile([1, SQC], f32, tag="ex2")
                        nc.scalar.mul(ex2[:], st1[:], 1.0 / D)
                        var = smallp.tile([1, SQC], f32, tag="var")
                        nc.vector.tensor_sub(var[:], ex2[:], msq[:])
                        std = smallp.tile([1, SQC], f32, tag="std")
                        nc.scalar.activation(std[:], var[:], AF.Sqrt, bias=epsb[:])
                        rstd = smallp.tile([1, SQC], f32, tag="rstd")
                        nc.vector.reciprocal(rstd[:], std[:])
                        mb = smallp.tile([P, SQC], f32, tag="mb")
                        nc.gpsimd.partition_broadcast(mb[:], nmean[:])
                        rb = smallp.tile([P, SQC], f32, tag="rb")
                        nc.gpsimd.partition_broadcast(rb[:], rstd[:])
                        for dc in range(DT):
                            nc.vector.tensor_add(xr[:, dc, :], xr[:, dc, :], mb[:])
                            nc.vector.tensor_mul(
                                xT[:, dc, c * SQC : (c + 1) * SQC],
                                xr[:, dc, :],
                                rb[:],
                            )

            # ---- final vocab projection (column-parallel, no collective) ----
            with (
                tc.tile_pool(name="fw", bufs=2) as fwp,
                tc.tile_pool(name="ob", bufs=4) as obp,
            ):
                for vc in range(NVC):
                    wv = fwp.tile([P, DT, VC], f32r, tag="wv")
                    nc.sync.dma_start(
                        out=wv[:],
                        in_=outw[:, vc * VC : (vc + 1) * VC].rearrange(
                            "(a p) m -> p a m", p=P
                        ),
                    )
                    for st in range(NT):
                        ps = psp.tile([P, SQC], f32, tag="mm")
                        for kt in range(DT):
                            nc.tensor.matmul(
                                ps[:, 0:VC],
                                lhsT=xT[:, kt, st * P : (st + 1) * P],
                                rhs=wv[:, kt, :],
                                start=(kt == 0),
                                stop=(kt == DT - 1),
                            )
                        ob = obp.tile([P, VC], f32, tag="ob")
                        nc.scalar.copy(ob[:], ps[:, 0:VC])
                        nc.sync.dma_start(
                            out=out[st * P : (st + 1) * P, vc * VC : (vc + 1) * VC],
                            in_=ob[:],
                        )
    nc.finalize()
    return nc


def _in_maps(tokens, emb, qw, kw, vw, ow, out_w):
    pe = _pos_encoding()
    peT = np.ascontiguousarray(pe.T)
    ident = np.eye(P, dtype=np.float32)
    j = np.arange(SQC)[None, :]
    i = np.arange(P)[:, None]
    masks = np.concatenate(
        [
            np.where(j >= P * trel + i, 0.0, NEG).astype(np.float32)
            for trel in range(4)
        ],
        axis=1,
    )
    masks = np.ascontiguousarray(masks)
    maps = []
    for c in range(8):
        g, r = divmod(c, NG)
        hc0 = r * HCOLS
        maps.append(
            {
                "tok": np.ascontiguousarray(
                    tokens[g].reshape(S, 1).astype(np.int32)
                ),
                "ident": ident,
                "onesc": np.ones((P, 1), dtype=np.float32),
                "masks": masks,
                "emb": emb,
                "peT": peT,
                "qw": np.ascontiguousarray(qw[:, :, hc0 : hc0 + HCOLS]),
                "kw": np.ascontiguousarray(kw[:, :, hc0 : hc0 + HCOLS]),
                "vw": np.ascontiguousarray(vw[:, :, hc0 : hc0 + HCOLS]),
                "ow": np.ascontiguousarray(ow[:, hc0 : hc0 + HCOLS, :]),
                "outw": np.ascontiguousarray(out_w[:, r * VS : (r + 1) * VS]),
            }
        )
    return maps


def run(inputs, trace=False):
    """Build+compile (cached), run on 8 cores, return (full_output, results)."""
    global _COMPILED
    from concourse.bass_utils import run_bass_kernel_spmd

    if _COMPILED is None:
        _COMPILED = _build()
    nc = _COMPILED

    tokens = np.asarray(inputs["tokens"])
    maps = _in_maps(
        np.asarray(tokens),
        np.ascontiguousarray(np.asarray(inputs["emb"], dtype=np.float32)),
        np.asarray(inputs["qw"], dtype=np.float32),
        np.asarray(inputs["kw"], dtype=np.float32),
        np.asarray(inputs["vw"], dtype=np.float32),
        np.asarray(inputs["ow"], dtype=np.float32),
        np.ascontiguousarray(np.asarray(inputs["out_w"], dtype=np.float32)),
    )
    res = run_bass_kernel_spmd(nc, maps, core_ids=list(range(8)), trace=trace)
    full = np.empty((B, S, V), dtype=np.float32)
    for c in range(8):
        g, r = divmod(c, NG)
        full[g, :, r * VS : (r + 1) * VS] = res.results[c]["out"]
    return full, res


def kernel(**inputs):
    full, _ = run(inputs)
    return full


# revision 18
# speedup vs baseline: 1.2173x; 1.0370x over previous
"""Trainium2 Bass kernel for a 4-layer causal-attention LM.

Model: V=32000, D=1024, H=16 heads, L=4 layers, B=2, S=1024.
  x = emb[tokens] + pos_enc
  per layer: q,k,v = x@W; causal softmax attention; out-proj; residual; LN
  logits = x @ out_w

Sharding over 8 NeuronCores (per sharding hint):
  DP=2 over batch  x  Megatron TP=4 over heads.
  Core c: batch g=c//4, rank r=c%4 owns heads [4r, 4r+4) and vocab cols
  [8000r, 8000(r+1)). Attention/QKV column-parallel, out-proj row-parallel
  with a per-512-column-chunk AllReduce (pipelined). Final vocab projection
  is column-parallel; the host concatenates shards (no collective).

Layout: activations kept feature-major ("xT": [d partitions, seq free]) so
every matmul contracts over partitions with zero transposes (only the
embedding entry needs PE transposes). Matmuls run as float32r (FP22,
full PE rate); the exp/V attention operands run bf16. Softmax uses
transposed scores [sk, sq]; per-query sums come free from a ones column
appended to V in the A@V matmul; normalization folds into the ctx
eviction. LayerNorm stats (feature-axis) via ones-vector matmuls.
"""

import numpy as np

V, D, H, L = 32000, 1024, 16, 4
B, S = 2, 1024
HD = D // H            # 64
P = 128
NG = 4                 # TP degree (cores per batch group)
HL = H // NG           # 4 heads per core
HCOLS = HL * HD        # 256 projection cols per core
VS = V // NG           # 8000 vocab shard
DT = D // P            # 8 d-tiles
SQC = 512              # seq chunk for AR pipelining
NSQC = S // SQC        # 2
NT = S // P            # 8 seq tiles
VC = 500               # vocab tile (8000 = 16*500)
NVC = VS // VC         # 16
SCALE = 1.0 / float(np.sqrt(HD))
EPS = 1e-5
NEG = -1.0e9
RG = [[0, 1, 2, 3], [4, 5, 6, 7]]

_COMPILED = None  # cache (nc) across calls


def _pos_encoding():
    pos = np.arange(S, dtype=np.float32)[:, None]
    div = np.exp(np.arange(0, D, 2, dtype=np.float32) * (-np.log(10000.0) / D))
    ang = pos * div
    pe = np.stack([np.sin(ang), np.cos(ang)], axis=-1).reshape(S, D)
    return pe.astype(np.float32)


def _build():
    import concourse.bass as bass
    import concourse.tile as tile
    from concourse import bacc, mybir

    f32 = mybir.dt.float32
    f32r = mybir.dt.float32r
    bf16 = mybir.dt.bfloat16
    i32 = mybir.dt.int32
    AF = mybir.ActivationFunctionType

    nc = bacc.Bacc("TRN2", target_bir_lowering=False, debug=False, num_devices=8)

    tok = nc.dram_tensor("tok", [S, 1], i32, kind="ExternalInput").ap()
    ident_d = nc.dram_tensor("ident", [P, P], f32, kind="ExternalInput").ap()
    ones_d = nc.dram_tensor("onesc", [P, 1], f32r, kind="ExternalInput").ap()
    masks_d = nc.dram_tensor("masks", [P, 4 * SQC], f32, kind="ExternalInput").ap()
    emb = nc.dram_tensor("emb", [V, D], f32, kind="ExternalInput").ap()
    peT = nc.dram_tensor("peT", [D, S], f32, kind="ExternalInput").ap()
    qw = nc.dram_tensor("qw", [L, D, HCOLS], f32r, kind="ExternalInput").ap()
    kw = nc.dram_tensor("kw", [L, D, HCOLS], f32r, kind="ExternalInput").ap()
    vw = nc.dram_tensor("vw", [L, D, HCOLS], f32r, kind="ExternalInput").ap()
    ow = nc.dram_tensor("ow", [L, HCOLS, D], f32r, kind="ExternalInput").ap()
    outw = nc.dram_tensor("outw", [D, VS], f32r, kind="ExternalInput").ap()
    out = nc.dram_tensor("out", [S, VS], f32, kind="ExternalOutput").ap()

    with tile.TileContext(nc) as tc:
        with (
            tc.tile_pool(name="const", bufs=1) as constp,
            tc.tile_pool(name="xp", bufs=1) as xp,
            tc.tile_pool(name="psum", bufs=2, space="PSUM") as psp,
        ):
            # ---- constants (host-provided: walrus chokes on affine_select) ----
            ident = constp.tile([P, P], f32)
            nc.sync.dma_start(out=ident[:], in_=ident_d[:])
            ones = constp.tile([P, 1], f32r)
            nc.sync.dma_start(out=ones[:], in_=ones_d[:])
            epsb = constp.tile([1, 1], f32)
            nc.vector.memset(epsb[:], EPS)
            # additive causal masks for the 4 diagonal sk-tiles of each sq
            # chunk: mask[trel][i, j] = 0 if j >= 128*trel + i else NEG
            masks = constp.tile([P, 4, SQC], f32)
            nc.sync.dma_start(
                out=masks[:], in_=masks_d.rearrange("p (t s) -> p t s", t=4)
            )

            # persistent activations, feature-major: x[d, s], d = a*128 + p
            xT = xp.tile([P, DT, S], f32r)

            # ---- embedding: gather rows, transpose to feature-major, +pe ----
            with tc.tile_pool(name="embp", bufs=2) as embp:
                tokt = embp.tile([P, NT], i32, bufs=1)
                nc.sync.dma_start(
                    out=tokt[:], in_=tok.rearrange("(t p) o -> p (t o)", p=P)
                )
                for st in range(NT):
                    xrow = embp.tile([P, D], f32, tag="xrow")
                    nc.gpsimd.indirect_dma_start(
                        out=xrow[:],
                        out_offset=None,
                        in_=emb[:],
                        in_offset=bass.IndirectOffsetOnAxis(
                            ap=tokt[:, st : st + 1], axis=0
                        ),
                    )
                    pesb = embp.tile([P, DT, P], f32, tag="pesb")
                    nc.sync.dma_start(
                        out=pesb[:],
                        in_=peT[:, st * P : (st + 1) * P].rearrange(
                            "(a p) s -> p a s", p=P
                        ),
                    )
                    for dc in range(DT):
                        tps = psp.tile([P, P], f32, tag="mm", name=f"tps_{st}_{dc}")
                        nc.tensor.transpose(
                            tps[:], xrow[:, dc * P : (dc + 1) * P], ident[:]
                        )
                        nc.vector.tensor_add(
                            xT[:, dc, st * P : (st + 1) * P],
                            tps[:],
                            pesb[:, dc, :],
                        )

            # ---- transformer layers ----
            with (
                tc.tile_pool(name="wp", bufs=4) as wp,
                tc.tile_pool(name="owp", bufs=2) as owp,
                tc.tile_pool(name="apl", bufs=1) as apool,
                tc.tile_pool(name="expp", bufs=4) as expp,
                tc.tile_pool(name="lnp", bufs=1) as lnp,
                tc.tile_pool(name="dcp", bufs=2) as dcp,
                tc.tile_pool(name="small", bufs=1) as smallp,
                tc.tile_pool(name="dram", bufs=2, space="DRAM") as dramp,
            ):
                for l in range(L):
                    qw_sb = wp.tile([P, DT, HCOLS], f32r, tag="w", name=f"qw{l}")
                    nc.sync.dma_start(
                        out=qw_sb[:], in_=qw[l].rearrange("(a p) m -> p a m", p=P)
                    )
                    kw_sb = wp.tile([P, DT, HCOLS], f32r, tag="w", name=f"kw{l}")
                    nc.sync.dma_start(
                        out=kw_sb[:], in_=kw[l].rearrange("(a p) m -> p a m", p=P)
                    )
                    vw_sb = wp.tile([P, DT, HCOLS], f32r, tag="w", name=f"vw{l}")
                    nc.sync.dma_start(
                        out=vw_sb[:], in_=vw[l].rearrange("(a p) m -> p a m", p=P)
                    )
                    ow_sb = owp.tile([P, 2, D], f32r, tag="ow", name=f"ow{l}")
                    nc.sync.dma_start(
                        out=ow_sb[:], in_=ow[l].rearrange("(a p) m -> p a m", p=P)
                    )

                    # q,k feature-major [headcol, s]; head h: partitions
                    # 64*(h%2).., chunk h//2
                    qT = apool.tile([P, 2, S], f32r, tag="qT", name=f"qT{l}")
                    kT = apool.tile([P, 2, S], f32r, tag="kT", name=f"kT{l}")
                    for dst, wsb in ((qT, qw_sb), (kT, kw_sb)):
                        for hp in range(2):
                            for c in range(NSQC):
                                ps = psp.tile([P, SQC], f32, tag="mm")
                                for kt in range(DT):
                                    nc.tensor.matmul(
                                        ps[:],
                                        lhsT=wsb[:, kt, hp * P : (hp + 1) * P],
                                        rhs=xT[:, kt, c * SQC : (c + 1) * SQC],
                                        start=(kt == 0),
                                        stop=(kt == DT - 1),
                                    )
                                nc.scalar.copy(
                                    dst[:, hp, c * SQC : (c + 1) * SQC], ps[:]
                                )

                    # v seq-major [s, headcol] bf16, with ones column at 64
                    vS = apool.tile([P, NT, HL, 66], bf16, tag="vS", name=f"vS{l}")
                    for st in range(NT):
                        nc.vector.memset(vS[:, st, :, 64:65], 1.0)
                    for st in range(NT):
                        ps = psp.tile([P, HCOLS], f32, tag="mm")
                        for kt in range(DT):
                            nc.tensor.matmul(
                                ps[:],
                                lhsT=xT[:, kt, st * P : (st + 1) * P],
                                rhs=vw_sb[:, kt, :],
                                start=(kt == 0),
                                stop=(kt == DT - 1),
                            )
                        nc.scalar.copy(
                            vS[:, st, :, 0:64],
                            ps[:].rearrange("p (h e) -> p h e", h=HL),
                        )

                    # ---- attention (transposed scores), ctx feature-major ----
                    ctx = apool.tile([P, 2, S], f32r, tag="ctx", name=f"ctx{l}")
                    for h in range(HL):
                        hp, hr = divmod(h, 2)
                        p0 = 64 * hr
                        for c in range(NSQC):
                            nt_vis = 4 * c + 4
                            av = psp.tile([P, SQC], f32, tag="av")
                            for t in range(nt_vis):
                                sc = psp.tile([P, SQC], f32, tag="sc")
                                nc.tensor.matmul(
                                    sc[:],
                                    lhsT=kT[p0 : p0 + 64, hp, t * P : (t + 1) * P],
                                    rhs=qT[p0 : p0 + 64, hp, c * SQC : (c + 1) * SQC],
                                    start=True,
                                    stop=True,
                                )
                                trel = t - 4 * c
                                if trel >= 0:
                                    nc.vector.tensor_add(
                                        sc[:], sc[:], masks[:, trel, :]
                                    )
                                ex = expp.tile([P, SQC], bf16, tag="ex")
                                nc.scalar.activation(
                                    ex[:], sc[:], AF.Exp, scale=SCALE
                                )
                                nc.tensor.matmul(
                                    av[0:65, :],
                                    lhsT=vS[:, t, h, 0:65],
                                    rhs=ex[:],
                                    start=(t == 0),
                                    stop=(t == nt_vis - 1),
                                )
                            ssum = smallp.tile([1, SQC], f32, tag="ssum")
                            nc.scalar.copy(ssum[:], av[64:65, :])
                            inv = smallp.tile([1, SQC], f32, tag="inv")
                            nc.vector.reciprocal_approx_fast(inv[:], ssum[:])
                            invb = smallp.tile([64, SQC], f32, tag="invb")
                            nc.gpsimd.partition_broadcast(invb[:], inv[:])
                            nc.vector.tensor_mul(
                                ctx[p0 : p0 + 64, hp, c * SQC : (c + 1) * SQC],
                                av[0:64, :],
                                invb[:],
                            )

                    # ---- out-proj partial + AR + residual + LN (per chunk) ----
                    for c in range(NSQC):
                        ar_in = dramp.tile(
                            [D, SQC], bf16, tag="arin", name=f"ari{l}_{c}"
                        )
                        for dc in range(DT):
                            ps = psp.tile([P, SQC], f32, tag="mm")
                            for kt in range(2):
                                nc.tensor.matmul(
                                    ps[:],
                                    lhsT=ow_sb[:, kt, dc * P : (dc + 1) * P],
                                    rhs=ctx[:, kt, c * SQC : (c + 1) * SQC],
                                    start=(kt == 0),
                                    stop=(kt == 1),
                                )
                            po = dcp.tile([P, SQC], bf16, tag="po")
                            nc.scalar.copy(po[:], ps[:])
                            nc.sync.dma_start(
                                out=ar_in[dc * P : (dc + 1) * P, :], in_=po[:]
                            )
                        ar_out = dramp.tile(
                            [D, SQC], bf16, tag="arout", name=f"aro{l}_{c}"
                        )
                        nc.gpsimd.collective_compute(
                            "AllReduce",
                            mybir.AluOpType.add,
                            replica_groups=RG,
                            ins=[ar_in[:].opt()],
                            outs=[ar_out[:].opt()],
                        )
                        # residual + stats + normalize (in place into xT)
                        xr = lnp.tile([P, DT, SQC], f32r, tag="xr")
                        st0 = psp.tile([1, SQC], f32, tag="av")
                        st1 = psp.tile([1, SQC], f32, tag="sc")
                        for dc in range(DT):
                            arL = dcp.tile([P, SQC], bf16, tag="arL")
                            nc.sync.dma_start(
                                out=arL[:], in_=ar_out[dc * P : (dc + 1) * P, :]
                            )
                            nc.vector.tensor_add(
                                xr[:, dc, :],
                                arL[:],
                                xT[:, dc, c * SQC : (c + 1) * SQC],
                            )
                            sqt = dcp.tile([P, SQC], f32r, tag="sqt")
                            nc.scalar.activation(sqt[:], xr[:, dc, :], AF.Square)
                            nc.tensor.matmul(
                                st0[:],
                                lhsT=ones[:],
                                rhs=xr[:, dc, :],
                                start=(dc == 0),
                                stop=(dc == DT - 1),
                            )
                            nc.tensor.matmul(
                                st1[:],
                                lhsT=ones[:],
                                rhs=sqt[:],
                                start=(dc == 0),
                                stop=(dc == DT - 1),
                            )
                        mean = smallp.tile([1, SQC], f32, tag="mean")
                        nc.scalar.mul(mean[:], st0[:], 1.0 / D)
                        nmean = smallp.tile([1, SQC], f32, tag="nmean")
                        nc.scalar.mul(nmean[:], mean[:], -1.0)
                        msq = smallp.tile([1, SQC], f32, tag="msq")
                        nc.scalar.activation(msq[:], mean[:], AF.Square)
                        ex2 = smallp.tile([1, SQC], f32, tag="ex2")
                        nc.scalar.mul(ex2[:], st1[:], 1.0 / D)
                        var = smallp.tile([1, SQC], f32, tag="var")
                        nc.vector.tensor_sub(var[:], ex2[:], msq[:])
                        std = smallp.tile([1, SQC], f32, tag="std")
                        nc.scalar.activation(std[:], var[:], AF.Sqrt, bias=epsb[:])
                        rstd = smallp.tile([1, SQC], f32, tag="rstd")
                        nc.vector.reciprocal_approx_fast(rstd[:], std[:])
                        mb = smallp.tile([P, SQC], f32, tag="mb")
                        nc.gpsimd.partition_broadcast(mb[:], nmean[:])
                        rb = smallp.tile([P, SQC], f32, tag="rb")
                        nc.gpsimd.partition_broadcast(rb[:], rstd[:])
                        for dc in range(DT):
                            nc.vector.tensor_add(xr[:, dc, :], xr[:, dc, :], mb[:])
                            nc.vector.tensor_mul(
                                xT[:, dc, c * SQC : (c + 1) * SQC],
                                xr[:, dc, :],
                                rb[:],
                            )

            # ---- final vocab projection (column-parallel, no collective) ----
            with (
                tc.tile_pool(name="fw", bufs=2) as fwp,
                tc.tile_pool(name="ob", bufs=4) as obp,
            ):
                for vc in range(NVC):
                    wv = fwp.tile([P, DT, VC], f32r, tag="wv")
                    nc.sync.dma_start(
                        out=wv[:],
                        in_=outw[:, vc * VC : (vc + 1) * VC].rearrange(
                            "(a p) m -> p a m", p=P
                        ),
                    )
                    for st in range(NT):
                        ps = psp.tile([P, SQC], f32, tag="mm")
                        for kt in range(DT):
                            nc.tensor.matmul(
                                ps[:, 0:VC],
                                lhsT=xT[:, kt, st * P : (st + 1) * P],
                                rhs=wv[:, kt, :],
                                start=(kt == 0),
                                stop=(kt == DT - 1),
                            )
                        ob = obp.tile([P, VC], f32, tag="ob")
                        nc.scalar.copy(ob[:], ps[:, 0:VC])
                        nc.sync.dma_start(
                            out=out[st * P : (st + 1) * P, vc * VC : (vc + 1) * VC],
                            in_=ob[:],
                        )
    nc.finalize()
    return nc


def _in_maps(tokens, emb, qw, kw, vw, ow, out_w):
    pe = _pos_encoding()
    peT = np.ascontiguousarray(pe.T)
    ident = np.eye(P, dtype=np.float32)
    j = np.arange(SQC)[None, :]
    i = np.arange(P)[:, None]
    masks = np.concatenate(
        [
            np.where(j >= P * trel + i, 0.0, NEG).astype(np.float32)
            for trel in range(4)
        ],
        axis=1,
    )
    masks = np.ascontiguousarray(masks)
    maps = []
    for c in range(8):
        g, r = divmod(c, NG)
        hc0 = r * HCOLS
        maps.append(
            {
                "tok": np.ascontiguousarray(
                    tokens[g].reshape(S, 1).astype(np.int32)
                ),
                "ident": ident,
                "onesc": np.ones((P, 1), dtype=np.float32),
                "masks": masks,
                "emb": emb,
                "peT": peT,
                "qw": np.ascontiguousarray(qw[:, :, hc0 : hc0 + HCOLS]),
                "kw": np.ascontiguousarray(kw[:, :, hc0 : hc0 + HCOLS]),
                "vw": np.ascontiguousarray(vw[:, :, hc0 : hc0 + HCOLS]),
                "ow": np.ascontiguousarray(ow[:, hc0 : hc0 + HCOLS, :]),
                "outw": np.ascontiguousarray(out_w[:, r * VS : (r + 1) * VS]),
            }
        )
    return maps


def run(inputs, trace=False):
    """Build+compile (cached), run on 8 cores, return (full_output, results)."""
    global _COMPILED
    from concourse.bass_utils import run_bass_kernel_spmd

    if _COMPILED is None:
        _COMPILED = _build()
    nc = _COMPILED

    tokens = np.asarray(inputs["tokens"])
    maps = _in_maps(
        np.asarray(tokens),
        np.ascontiguousarray(np.asarray(inputs["emb"], dtype=np.float32)),
        np.asarray(inputs["qw"], dtype=np.float32),
        np.asarray(inputs["kw"], dtype=np.float32),
        np.asarray(inputs["vw"], dtype=np.float32),
        np.asarray(inputs["ow"], dtype=np.float32),
        np.ascontiguousarray(np.asarray(inputs["out_w"], dtype=np.float32)),
    )
    res = run_bass_kernel_spmd(nc, maps, core_ids=list(range(8)), trace=trace)
    full = np.empty((B, S, V), dtype=np.float32)
    for c in range(8):
        g, r = divmod(c, NG)
        full[g, :, r * VS : (r + 1) * VS] = res.results[c]["out"]
    return full, res


def kernel(**inputs):
    full, _ = run(inputs)
    return full
